# revision 20
# baseline (speedup 1.0000x reference)
"""Trainium2 Bass kernel for nn_AttentionPropagation (sparse attention propagation).

Reference computation:
  Q = cat(dense_xyz, dense_feat) @ Wq.T + bq            [B, N2, F]
  K = cat(sparse_xyz, sparse_feat) @ Wk.T + bk          [B, N1, F]
  V = sparse_feat @ Wv.T + bv                           [B, N1, F]
  attn = softmax(Q K^T / sqrt(F) - 0.5 * dist(dense_xyz, sparse_xyz))
  out = (attn @ V + dense_feat) @ Wo.T + bo             [B, N2, F]

Shapes: B=2, N1=4096 (sparse/keys), N2=32768 (dense/queries), F=128.

Sharding: queries (N2) split across 8 cores; sparse K/V + weights replicated.

Per-core kernel works entirely in a transposed layout (keys on partitions,
queries on the free dim) so that attn @ V needs no on-chip transposes:
  st[k, q]  = K @ Q^T            (PE, chunked 128 keys x 512 queries)
  ds[k, q]  = 0.25 * dist^2      (PE, K=5 matmul of augmented xyz vectors)
  hd        = sqrt(ds + eps)     (ACT)  == 0.5 * dist
  spre      = st - hd            (DVE, written fp16)
  attn      = exp(spre)          (ACT, in place; no max-subtraction needed:
                                  logits are bounded in roughly [-7, 2])
  P^T[f, q] = sum_k V[k, f] attn[k, q]      (PE, fp16)
  sums[q]   = sum_k attn[k, q]              (PE, ones-vector matmul)
  X^T       = P^T / sums + bv + dense_feat^T
  out^T     = Wo @ X^T + bo                 (PE)
Host transposes out^T back and concatenates the 8 query shards.

sqrt and exp live in different ACT table sets (~2.7us per switch), so the
pipeline runs phase A (all sqrt) / phase B (all exp) over units of 2 query
groups to amortize table loads.
"""

import os
import numpy as np

os.environ.setdefault("JAX_COMPILATION_CACHE_DIR", "/tmp/jax_bass_cache")
os.environ.setdefault("JAX_PERSISTENT_CACHE_MIN_ENTRY_SIZE_BYTES", "0")
os.environ.setdefault("JAX_PERSISTENT_CACHE_MIN_COMPILE_TIME_SECS", "1")

import concourse.bacc as bacc
import concourse.tile as tile
import concourse.mybir as mybir
from concourse import bass_utils
from concourse.tile import add_dep_helper

F32 = mybir.dt.float32
F32R = mybir.dt.float32r
F16 = mybir.dt.float16
AF = mybir.ActivationFunctionType
OP = mybir.AluOpType

B = 2
N1 = 4096          # sparse points (keys)
N2 = 32768         # dense points (queries)
FEAT = 128
SCALE = FEAT ** -0.5
NCORES = 8
QPC = N2 // NCORES  # queries per core per batch (4096)
QG = 512            # query group (matmul moving free dim)
GROUPS = QPC // QG  # 8 groups per batch
KC = 128            # key chunk (PSUM partition dim)
CHUNKS = N1 // KC   # 32
PAIRS = CHUNKS // 2  # 16 chunk pairs (ds/st psum tiles hold 2 chunks)
UNIT = 2            # query groups per sqrt/exp table-switch unit
NAUG = 16           # hi/lo-split dist^2 augmentation rows
SQRT_EPS = 1e-5     # covers residual fp16-split ds error (~3e-6) and the
                    # reference's 1e-12 clamp; adds <2e-3 logit error

_NC_CACHE = {}


def _build():
    if "nc" in _NC_CACHE:
        return _NC_CACHE["nc"]
    nc = bacc.Bacc("TRN2", target_bir_lowering=False, debug=False)

    # ---- DRAM I/O (per-core shard) ----
    dfT = nc.dram_tensor("dfT", [B, FEAT, QPC], F32R, kind="ExternalInput")
    dxT = nc.dram_tensor("dxT", [B, 3, QPC], F32R, kind="ExternalInput")
    qaug = nc.dram_tensor("qaug", [B, NAUG, QPC], F16, kind="ExternalInput")
    sfT = nc.dram_tensor("sfT", [B, FEAT, N1], F32R, kind="ExternalInput")
    sxT = nc.dram_tensor("sxT", [B, 3, N1], F32R, kind="ExternalInput")
    kaug = nc.dram_tensor("kaug", [B, NAUG, N1], F16, kind="ExternalInput")
    WqfT = nc.dram_tensor("WqfT", [FEAT, FEAT], F32R, kind="ExternalInput")
    WqxT = nc.dram_tensor("WqxT", [3, FEAT], F32R, kind="ExternalInput")
    WkfT = nc.dram_tensor("WkfT", [FEAT, FEAT], F32R, kind="ExternalInput")
    WkxT = nc.dram_tensor("WkxT", [3, FEAT], F32R, kind="ExternalInput")
    WvT = nc.dram_tensor("WvT", [FEAT, FEAT], F32R, kind="ExternalInput")
    WoT = nc.dram_tensor("WoT", [FEAT, FEAT], F16, kind="ExternalInput")
    bq = nc.dram_tensor("bq", [FEAT, 1], F32, kind="ExternalInput")
    bk = nc.dram_tensor("bk", [FEAT, 1], F32, kind="ExternalInput")
    bv = nc.dram_tensor("bv", [FEAT, 1], F32, kind="ExternalInput")
    bo = nc.dram_tensor("bo", [FEAT, 1], F32, kind="ExternalInput")
    outT = nc.dram_tensor("outT", [B, FEAT, QPC], F32, kind="ExternalOutput")

    with tile.TileContext(nc) as tc:
        with tc.tile_pool(name="const", bufs=1) as const_p, \
             tc.tile_pool(name="batch", bufs=1) as batch_p, \
             tc.tile_pool(name="slab", bufs=2) as slab_p, \
             tc.tile_pool(name="hd", bufs=3) as hd_p, \
             tc.tile_pool(name="gsmall", bufs=3) as gsm_p, \
             tc.tile_pool(name="gout", bufs=2) as gout_p, \
             tc.tile_pool(name="ps_ds", bufs=2, space="PSUM") as ps_ds, \
             tc.tile_pool(name="ps_st", bufs=2, space="PSUM") as ps_st, \
             tc.tile_pool(name="ps_pt", bufs=1, space="PSUM") as ps_pt, \
             tc.tile_pool(name="ps_sm", bufs=1, space="PSUM") as ps_sm:

            # ---- constants ----
            wqf_t = const_p.tile([FEAT, FEAT], F32R)
            wqx_t = const_p.tile([3, FEAT], F32R)
            wkf_t = const_p.tile([FEAT, FEAT], F32R)
            wkx_t = const_p.tile([3, FEAT], F32R)
            wv_t = const_p.tile([FEAT, FEAT], F32R)
            wo_t = const_p.tile([FEAT, FEAT], F16)
            bq_t = const_p.tile([FEAT, 1], F32)
            bk_t = const_p.tile([FEAT, 1], F32)
            bv_t = const_p.tile([FEAT, 1], F32)
            bo_t = const_p.tile([FEAT, 1], F32)
            ones16 = const_p.tile([KC, 1], F16)
            eps_t = const_p.tile([KC, 1], F32)
            nc.vector.memset(eps_t, SQRT_EPS)
            for t, d in ((wqf_t, WqfT), (wqx_t, WqxT), (wkf_t, WkfT),
                         (wkx_t, WkxT), (wv_t, WvT), (wo_t, WoT),
                         (bq_t, bq), (bk_t, bk), (bv_t, bv), (bo_t, bo)):
                nc.sync.dma_start(out=t, in_=d.ap())
            nc.vector.memset(ones16, 1.0)

            # ACT runs exactly two table-based funcs (Sqrt, Exp) in different
            # table sets; a table switch costs ~2.7us.  The scheduler will
            # happily interleave units' sqrt/exp streams, so pin every sqrt
            # of unit u+1 behind the last exp of unit u.
            last_exp = [None]
            last_sqrt = [None]

            for b in range(B):
                # ---- batch setup: load sparse side, project K^T and V ----
                sf_t = batch_p.tile([FEAT, N1], F32R, tag="sf")
                sx_t = batch_p.tile([3, N1], F32R, tag="sx")
                ka_t = batch_p.tile([NAUG, N1], F16, tag="ka")
                nc.sync.dma_start(out=sf_t, in_=sfT.ap()[b])
                nc.sync.dma_start(out=sx_t, in_=sxT.ap()[b])
                nc.sync.dma_start(out=ka_t, in_=kaug.ap()[b])

                kt_t = batch_p.tile([FEAT, N1], F16, tag="kt")   # K^T slab
                v_t = batch_p.tile([KC, CHUNKS * FEAT], F16, tag="v")  # V slab
                for j in range(N1 // QG):
                    ps = ps_st.tile([KC, QG], F32, tag="st")
                    nc.tensor.matmul(ps, wkf_t,
                                     sf_t[:, j * QG:(j + 1) * QG],
                                     start=True, stop=False)
                    nc.tensor.matmul(ps, wkx_t,
                                     sx_t[:, j * QG:(j + 1) * QG],
                                     start=False, stop=True)
                    nc.vector.tensor_scalar_add(
                        kt_t[:, j * QG:(j + 1) * QG], ps, bk_t)
                for c in range(CHUNKS):
                    ps = ps_st.tile([KC, QG], F32, tag="st")
                    nc.tensor.matmul(ps[:, 0:FEAT],
                                     sf_t[:, c * KC:(c + 1) * KC], wv_t,
                                     start=True, stop=True)
                    nc.vector.tensor_copy(
                        v_t[:, c * FEAT:(c + 1) * FEAT], ps[:, 0:FEAT])

                for u in range(GROUPS // UNIT):
                    gs = [u * UNIT + i for i in range(UNIT)]
                    spre = {}
                    dfg = {}
                    # ---------- phase A: ds -> sqrt -> st -> subtract ----------
                    for g in gs:
                        q0 = g * QG
                        df_t = gsm_p.tile([FEAT, QG], F32R, tag="df")
                        dx_t = gsm_p.tile([3, QG], F32R, tag="dx")
                        qa_t = gsm_p.tile([NAUG, QG], F16, tag="qa")
                        nc.sync.dma_start(out=df_t, in_=dfT.ap()[b, :, q0:q0 + QG])
                        nc.sync.dma_start(out=dx_t, in_=dxT.ap()[b, :, q0:q0 + QG])
                        nc.sync.dma_start(out=qa_t, in_=qaug.ap()[b, :, q0:q0 + QG])
                        dfg[g] = df_t

                        ps_q = ps_ds.tile([KC, 2 * QG], F32, tag="ds")
                        nc.tensor.matmul(ps_q[:, 0:QG], wqf_t, df_t,
                                         start=True, stop=False)
                        nc.tensor.matmul(ps_q[:, 0:QG], wqx_t, dx_t,
                                         start=False, stop=True)
                        qt_t = gsm_p.tile([FEAT, QG], F16, tag="qt")
                        nc.vector.tensor_scalar_add(qt_t, ps_q[:, 0:QG], bq_t)

                        sp = slab_p.tile([KC, CHUNKS * QG], F16, tag="spre")
                        spre[g] = sp
                        for p in range(PAIRS):
                            c0, c1 = 2 * p, 2 * p + 1
                            pd = ps_ds.tile([KC, 2 * QG], F32, tag="ds")
                            nc.tensor.matmul(pd[:, 0:QG],
                                             ka_t[:, c0 * KC:(c0 + 1) * KC],
                                             qa_t, start=True, stop=True)
                            nc.tensor.matmul(pd[:, QG:2 * QG],
                                             ka_t[:, c1 * KC:(c1 + 1) * KC],
                                             qa_t, start=True, stop=True)
                            hd_t = hd_p.tile([KC, 2 * QG], F32, tag="hd")
                            sq_i = nc.scalar.activation(hd_t, pd, AF.Sqrt,
                                                        bias=eps_t[:, 0:1])
                            if last_exp[0] is not None:
                                add_dep_helper(sq_i.ins, last_exp[0],
                                               reason="ACT table phase order")
                            last_sqrt[0] = sq_i.ins

                            for ci, c in ((0, c0), (1, c1)):
                                pst = ps_st.tile([KC, QG], F32, tag="st")
                                nc.tensor.matmul(pst,
                                                 kt_t[:, c * KC:(c + 1) * KC],
                                                 qt_t, start=True, stop=True)
                                nc.vector.tensor_tensor(
                                    out=sp[:, c * QG:(c + 1) * QG],
                                    in0=pst,
                                    in1=hd_t[:, ci * QG:(ci + 1) * QG],
                                    op=OP.subtract)

                    # ---------- phase B: exp -> attn@V -> normalize -> out ----------
                    for g in gs:
                        q0 = g * QG
                        sp = spre[g]
                        pt = ps_pt.tile([FEAT, QG], F32, tag="pt")
                        sm = ps_sm.tile([KC, QG], F32, tag="sm")
                        for e in range(4):  # exp over 8 chunks at a time
                            lo, hi = e * 8 * QG, (e + 1) * 8 * QG
                            exp_i = nc.scalar.activation(sp[:, lo:hi],
                                                         sp[:, lo:hi], AF.Exp)
                            if last_sqrt[0] is not None:
                                add_dep_helper(exp_i.ins, last_sqrt[0],
                                               reason="ACT table phase order")
                            last_exp[0] = exp_i.ins
                            for c in range(e * 8, (e + 1) * 8):
                                nc.tensor.matmul(
                                    pt, v_t[:, c * FEAT:(c + 1) * FEAT],
                                    sp[:, c * QG:(c + 1) * QG],
                                    start=(c == 0), stop=(c == CHUNKS - 1))
                                nc.tensor.matmul(
                                    sm[0:1, :], ones16,
                                    sp[:, c * QG:(c + 1) * QG],
                                    start=(c == 0), stop=(c == CHUNKS - 1))
                        rs_t = gout_p.tile([1, QG], F32, tag="rs")
                        nc.vector.reciprocal(rs_t, sm[0:1, :])
                        rb_t = gout_p.tile([KC, QG], F32, tag="rb")
                        nc.gpsimd.partition_broadcast(rb_t, rs_t, channels=KC)
                        x1_t = gout_p.tile([FEAT, QG], F32, tag="x1")
                        nc.vector.tensor_tensor(out=x1_t, in0=pt, in1=rb_t,
                                                op=OP.mult)
                        xt_t = gout_p.tile([FEAT, QG], F16, tag="xt")
                        nc.vector.scalar_tensor_tensor(
                            out=xt_t, in0=x1_t, scalar=bv_t,
                            in1=dfg[g].bitcast(F32), op0=OP.add, op1=OP.add)
                        po = ps_sm.tile([KC, QG], F32, tag="sm")
                        nc.tensor.matmul(po[0:FEAT, :], wo_t, xt_t,
                                         start=True, stop=True)
                        o_t = gout_p.tile([FEAT, QG], F32, tag="o")
                        nc.vector.tensor_scalar_add(o_t, po[0:FEAT, :], bo_t)
                        nc.sync.dma_start(out=outT.ap()[b, :, q0:q0 + QG], in_=o_t)

    nc.compile()
    _NC_CACHE["nc"] = nc
    return nc


def _prep_inputs(sparse_xyz, sparse_feat, dense_xyz, dense_feat,
                 Wq, bq, Wk, bk, Wv, bv, Wo, bo):
    """Host-side layout prep: transposes, weight folding, xyz augmentation."""
    f32 = np.float32
    Wq = Wq.astype(f32) * f32(SCALE)
    bq_s = bq.astype(f32) * f32(SCALE)

    dfT = np.ascontiguousarray(dense_feat.transpose(0, 2, 1), dtype=f32)
    dxT = np.ascontiguousarray(dense_xyz.transpose(0, 2, 1), dtype=f32)
    sfT = np.ascontiguousarray(sparse_feat.transpose(0, 2, 1), dtype=f32)
    sxT = np.ascontiguousarray(sparse_xyz.transpose(0, 2, 1), dtype=f32)

    # ds = sum_d kaug[d] * qaug[d] = 0.25 * dist^2, computed as an fp16
    # matmul.  Naive [qn, 1, -2q] x [1, kn, k] augmentation cancels
    # catastrophically once inputs are rounded (negative ds -> sqrt NaN), so
    # every value is split hi/lo into two fp16 parts; fp16 x fp16 products
    # are exact in the fp32 PSUM accumulator, leaving ~3e-6 total error.
    f16, f64 = np.float16, np.float64

    def hilo(x):
        hi = x.astype(f16)
        lo = (x - hi.astype(f64)).astype(f16)
        return hi, lo

    qn = np.sum(dense_xyz.astype(f64) ** 2, axis=-1)   # [B, N2]
    kn = np.sum(sparse_xyz.astype(f64) ** 2, axis=-1)  # [B, N1]
    qnh, qnl = hilo(qn)
    knh, knl = hilo(kn)
    qch, qcl = hilo(dxT.astype(f64))                   # [B, 3, N2] each
    kch, kcl = hilo(sxT.astype(f64))
    one2 = np.ones((B, 1, N2), f16)
    one1 = np.ones((B, 1, N1), f16)
    quart2 = np.full((B, 1, N2), 0.25, f16)
    qaug = np.concatenate(
        [0.25 * qnh[:, None, :].astype(f16), 0.25 * qnl[:, None, :].astype(f16),
         quart2, quart2,
         -0.5 * qch, -0.5 * qch, -0.5 * qcl, -0.5 * qcl], axis=1).astype(f16)
    kaug = np.concatenate(
        [one1, one1, knh[:, None, :], knl[:, None, :],
         kch, kcl, kch, kcl], axis=1).astype(f16)

    common = {
        "sfT": sfT, "sxT": sxT, "kaug": kaug,
        "WqfT": np.ascontiguousarray(Wq[:, 3:].T, f32),
        "WqxT": np.ascontiguousarray(Wq[:, :3].T, f32),
        "WkfT": np.ascontiguousarray(Wk[:, 3:].T.astype(f32)),
        "WkxT": np.ascontiguousarray(Wk[:, :3].T.astype(f32)),
        "WvT": np.ascontiguousarray(Wv.T.astype(f32)),
        "WoT": np.ascontiguousarray(Wo.T.astype(np.float16)),
        "bq": bq_s.reshape(FEAT, 1),
        "bk": bk.astype(f32).reshape(FEAT, 1),
        "bv": bv.astype(f32).reshape(FEAT, 1),
        "bo": bo.astype(f32).reshape(FEAT, 1),
    }
    in_maps = []
    for c in range(NCORES):
        sl = slice(c * QPC, (c + 1) * QPC)
        m = dict(common)
        m["dfT"] = np.ascontiguousarray(dfT[:, :, sl])
        m["dxT"] = np.ascontiguousarray(dxT[:, :, sl])
        m["qaug"] = np.ascontiguousarray(qaug[:, :, sl])
        in_maps.append(m)
    return in_maps


def run_sharded(in_maps, trace=False):
    nc = _build()
    kwargs = {}
    if trace:
        kwargs = {"trace": True}
    return bass_utils.run_bass_kernel_spmd(
        nc, in_maps, core_ids=list(range(NCORES)), **kwargs)


def kernel(sparse_xyz, sparse_feat, dense_xyz, dense_feat,
           Wq, bq, Wk, bk, Wv, bv, Wo, bo):
    in_maps = _prep_inputs(sparse_xyz, sparse_feat, dense_xyz, dense_feat,
                           Wq, bq, Wk, bk, Wv, bv, Wo, bo)
    res = run_sharded(in_maps, trace=bool(os.environ.get("BASS_KERNEL_TRACE")))
    out = np.empty((B, N2, FEAT), dtype=np.float32)
    for c in range(NCORES):
        out[:, c * QPC:(c + 1) * QPC, :] = \
            res.results[c]["outT"].transpose(0, 2, 1)
    if os.environ.get("BASS_KERNEL_TRACE"):
        print("HW exec time:", res.exec_time_ns, "ns")
    return out


# revision 41
# speedup vs baseline: 3981.4427x; 3981.4427x over previous
"""Trainium2 Bass kernel for nn_AttentionPropagation (sparse attention propagation).

Reference computation:
  Q = cat(dense_xyz, dense_feat) @ Wq.T + bq            [B, N2, F]
  K = cat(sparse_xyz, sparse_feat) @ Wk.T + bk          [B, N1, F]
  V = sparse_feat @ Wv.T + bv                           [B, N1, F]
  attn = softmax(Q K^T / sqrt(F) - 0.5 * dist(dense_xyz, sparse_xyz))
  out = (attn @ V + dense_feat) @ Wo.T + bo             [B, N2, F]

Shapes: B=2, N1=4096 (sparse/keys), N2=32768 (dense/queries), F=128.

Sharding: queries (N2) split across 8 cores; sparse K/V + weights replicated.

Per-core kernel works entirely in a transposed layout (keys on partitions,
queries on the free dim) so that attn @ V needs no on-chip transposes:
  st[k, q]  = K @ Q^T            (PE, chunked 128 keys x 512 queries)
  ds[k, q]  = 0.25 * dist^2      (PE, K=16 fp16 matmul of hi/lo-split
                                  augmented xyz vectors; exact products)
  hd        = sqrt(ds + eps)     (ACT)  == 0.5 * dist
  spre      = st - hd            (DVE, written fp16)
  attn      = exp(spre)          (ACT, in place; no max-subtraction needed:
                                  logits are bounded in roughly [-7, 2])
  P^T[f, q] = sum_k V[k, f] attn[k, q]      (PE, fp16)
  sums[q]   = sum_k attn[k, q]              (PE, ones-vector matmul)
  X^T       = P^T / sums + bv + dense_feat^T
  out^T     = Wo @ X^T + bo                 (PE)
Host transposes out^T back and concatenates the 8 query shards.

sqrt and exp live in different ACT table sets (~2.7us per switch), so the
pipeline runs phase A (all sqrt) / phase B (all exp) over units of 2 query
groups to amortize table loads.
"""

import os
import numpy as np

os.environ.setdefault("JAX_COMPILATION_CACHE_DIR", "/tmp/jax_bass_cache")
os.environ.setdefault("JAX_PERSISTENT_CACHE_MIN_ENTRY_SIZE_BYTES", "0")
os.environ.setdefault("JAX_PERSISTENT_CACHE_MIN_COMPILE_TIME_SECS", "1")

import concourse.bacc as bacc
import concourse.tile as tile
import concourse.mybir as mybir
from concourse import bass_utils
from concourse.tile import add_dep_helper

F32 = mybir.dt.float32
F32R = mybir.dt.float32r
F16 = mybir.dt.float16
AF = mybir.ActivationFunctionType
OP = mybir.AluOpType

B = 2
N1 = 4096          # sparse points (keys)
N2 = 32768         # dense points (queries)
FEAT = 128
SCALE = FEAT ** -0.5
NCORES = 8
QPC = N2 // NCORES  # queries per core per batch (4096)
QG = 512            # query group (matmul moving free dim)
GROUPS = QPC // QG  # 8 groups per batch
KC = 128            # key chunk (PSUM partition dim)
CHUNKS = N1 // KC   # 32
PAIRS = CHUNKS // 2  # 16 chunk pairs (ds/st psum tiles hold 2 chunks)
UNITS = [(0, 1), (2, 3), (4, 5), (6, 7)]  # groups per sqrt/exp table unit
PRE_PAIRS = 4       # chunk pairs of the next unit pre-sqrt'd each unit
NAUG = 16           # hi/lo-split dist^2 augmentation rows
SQRT_EPS = 1e-5     # covers residual fp16-split ds error (~3e-6) and the
                    # reference's 1e-12 clamp; adds <2e-3 logit error

_NC_CACHE = {}


def _build():
    if "nc" in _NC_CACHE:
        return _NC_CACHE["nc"]
    nc = bacc.Bacc("TRN2", target_bir_lowering=False, debug=False)

    # ---- DRAM I/O (per-core shard) ----
    dfT = nc.dram_tensor("dfT", [B, FEAT, QPC], F32R, kind="ExternalInput")
    dxT = nc.dram_tensor("dxT", [B, 3, QPC], F32R, kind="ExternalInput")
    qaug = nc.dram_tensor("qaug", [B, NAUG, QPC], F16, kind="ExternalInput")
    sfT = nc.dram_tensor("sfT", [B, FEAT, N1], F32R, kind="ExternalInput")
    sxT = nc.dram_tensor("sxT", [B, 3, N1], F32R, kind="ExternalInput")
    kaug = nc.dram_tensor("kaug", [B, NAUG, N1], F16, kind="ExternalInput")
    WqfT = nc.dram_tensor("WqfT", [FEAT, FEAT], F32R, kind="ExternalInput")
    WqxT = nc.dram_tensor("WqxT", [3, FEAT], F32R, kind="ExternalInput")
    WkfT = nc.dram_tensor("WkfT", [FEAT, FEAT], F32R, kind="ExternalInput")
    WkxT = nc.dram_tensor("WkxT", [3, FEAT], F32R, kind="ExternalInput")
    WvT = nc.dram_tensor("WvT", [FEAT, FEAT], F32R, kind="ExternalInput")
    WoT = nc.dram_tensor("WoT", [FEAT, FEAT], F16, kind="ExternalInput")
    bq = nc.dram_tensor("bq", [FEAT, 1], F32, kind="ExternalInput")
    bk = nc.dram_tensor("bk", [FEAT, 1], F32, kind="ExternalInput")
    bv = nc.dram_tensor("bv", [FEAT, 1], F32, kind="ExternalInput")
    bo = nc.dram_tensor("bo", [FEAT, 1], F32, kind="ExternalInput")
    outT = nc.dram_tensor("outT", [B, FEAT, QPC], F32, kind="ExternalOutput")

    with tile.TileContext(nc) as tc:
        with tc.tile_pool(name="const", bufs=1) as const_p, \
             tc.tile_pool(name="batch", bufs=1) as batch_p, \
             tc.tile_pool(name="slab", bufs=2) as slab_p, \
             tc.tile_pool(name="hd", bufs=3) as hd_p, \
             tc.tile_pool(name="gsmall", bufs=3) as gsm_p, \
             tc.tile_pool(name="gout", bufs=2) as gout_p, \
             tc.tile_pool(name="ps_ds", bufs=2, space="PSUM") as ps_ds, \
             tc.tile_pool(name="ps_st", bufs=2, space="PSUM") as ps_st, \
             tc.tile_pool(name="ps_pt", bufs=1, space="PSUM") as ps_pt, \
             tc.tile_pool(name="ps_sm", bufs=1, space="PSUM") as ps_sm:

            # ---- constants ----
            wqf_t = const_p.tile([FEAT, FEAT], F32R)
            wqx_t = const_p.tile([3, FEAT], F32R)
            wkf_t = const_p.tile([FEAT, FEAT], F32R)
            wkx_t = const_p.tile([3, FEAT], F32R)
            wv_t = const_p.tile([FEAT, FEAT], F32R)
            wo_t = const_p.tile([FEAT, FEAT], F16)
            bq_t = const_p.tile([FEAT, 1], F32)
            bk_t = const_p.tile([FEAT, 1], F32)
            bv_t = const_p.tile([FEAT, 1], F32)
            bo_t = const_p.tile([FEAT, 1], F32)
            ones16 = const_p.tile([KC, 1], F16)
            eps_t = const_p.tile([KC, 1], F32)
            nc.vector.memset(eps_t, SQRT_EPS)
            for t, d in ((wqf_t, WqfT), (wqx_t, WqxT), (wkf_t, WkfT),
                         (wkx_t, WkxT), (wv_t, WvT), (wo_t, WoT),
                         (bq_t, bq), (bk_t, bk), (bv_t, bv), (bo_t, bo)):
                nc.sync.dma_start(out=t, in_=d.ap())
            nc.vector.memset(ones16, 1.0)

            # ACT runs exactly two table-based funcs (Sqrt, Exp) in different
            # table sets; a table switch costs ~2.7us.  The scheduler will
            # happily interleave units' sqrt/exp streams, so pin every sqrt
            # of unit u+1 behind the last exp of unit u.
            last_exp = [None]
            last_sqrt = [None]

            for b in range(B):
                # ---- batch setup: load sparse side, project K^T and V ----
                # sparse-side staging borrows slab slots (freed before phase A)
                sf_t = slab_p.tile([FEAT, N1], F32R, tag="spre")
                sx_t = slab_p.tile([3, N1], F32R, tag="spre")
                ka_t = batch_p.tile([NAUG, N1], F16, tag="ka")
                nc.sync.dma_start(out=sf_t, in_=sfT.ap()[b])
                nc.sync.dma_start(out=sx_t, in_=sxT.ap()[b])
                nc.sync.dma_start(out=ka_t, in_=kaug.ap()[b])

                kt_t = batch_p.tile([FEAT, N1], F16, tag="kt")   # K^T slab
                v_t = batch_p.tile([KC, CHUNKS * FEAT], F16, tag="v")  # V slab
                for j in range(N1 // QG):
                    ps = ps_st.tile([KC, QG], F32, tag="st")
                    nc.tensor.matmul(ps, wkf_t,
                                     sf_t[:, j * QG:(j + 1) * QG],
                                     start=True, stop=False)
                    nc.tensor.matmul(ps, wkx_t,
                                     sx_t[:, j * QG:(j + 1) * QG],
                                     start=False, stop=True)
                    nc.vector.tensor_scalar_add(
                        kt_t[:, j * QG:(j + 1) * QG], ps, bk_t)
                for c in range(CHUNKS):
                    ps = ps_st.tile([KC, QG], F32, tag="st")
                    nc.tensor.matmul(ps[:, 0:FEAT],
                                     sf_t[:, c * KC:(c + 1) * KC], wv_t,
                                     start=True, stop=True)
                    nc.vector.tensor_copy(
                        v_t[:, c * FEAT:(c + 1) * FEAT], ps[:, 0:FEAT])

                pre_hd = {}   # (g, p) -> hd tile sqrt'd during previous unit
                pre_dma = {}  # g -> (df_t, dx_t, qa_t) issued early

                def do_sqrt(dst_ap, c, qa_t):
                    pd = ps_ds.tile([KC, QG], F32, tag="ds")
                    nc.tensor.matmul(pd, ka_t[:, c * KC:(c + 1) * KC],
                                     qa_t, start=True, stop=True)
                    sq_i = nc.scalar.activation(dst_ap, pd, AF.Sqrt,
                                                bias=eps_t[:, 0:1])
                    if last_exp[0] is not None:
                        add_dep_helper(sq_i.ins, last_exp[0],
                                       reason="ACT table phase order")
                    last_sqrt[0] = sq_i.ins

                def group_dmas(g):
                    q0 = g * QG
                    df_t = gsm_p.tile([FEAT, QG], F32R, tag="df", bufs=4)
                    dx_t = gsm_p.tile([3, QG], F32R, tag="dx")
                    qa_t = gsm_p.tile([NAUG, QG], F16, tag="qa")
                    nc.sync.dma_start(out=df_t, in_=dfT.ap()[b, :, q0:q0 + QG])
                    nc.sync.dma_start(out=dx_t, in_=dxT.ap()[b, :, q0:q0 + QG])
                    nc.sync.dma_start(out=qa_t, in_=qaug.ap()[b, :, q0:q0 + QG])
                    return df_t, dx_t, qa_t

                for ui, gs in enumerate(UNITS):
                    spre = {}
                    dfg = {}
                    # ---------- phase A: ds -> sqrt -> st -> subtract ----------
                    for g in gs:
                        df_t, dx_t, qa_t = pre_dma.pop(g, None) or group_dmas(g)

                        dfg[g] = df_t
                        ps_q = ps_ds.tile([KC, QG], F32, tag="ds")
                        nc.tensor.matmul(ps_q, wqf_t, df_t,
                                         start=True, stop=False)
                        nc.tensor.matmul(ps_q, wqx_t, dx_t,
                                         start=False, stop=True)
                        qt_t = gsm_p.tile([FEAT, QG], F16, tag="qt")
                        nc.vector.tensor_scalar_add(qt_t, ps_q, bq_t)

                        sp = slab_p.tile([KC, CHUNKS * QG], F16, tag="spre")
                        spre[g] = sp
                        for p in range(PAIRS):
                            c0, c1 = 2 * p, 2 * p + 1
                            hd_t = pre_hd.pop((g, p), None)
                            if hd_t is None:
                                hd_t = hd_p.tile([KC, 2 * QG], F32, tag="hd")
                                for ci, c in ((0, c0), (1, c1)):
                                    do_sqrt(hd_t[:, ci * QG:(ci + 1) * QG],
                                            c, qa_t)

                            pst = ps_st.tile([KC, 2 * QG], F32, tag="st")
                            nc.tensor.matmul(pst[:, 0:QG],
                                             kt_t[:, c0 * KC:(c0 + 1) * KC],
                                             qt_t, start=True, stop=True)
                            nc.tensor.matmul(pst[:, QG:2 * QG],
                                             kt_t[:, c1 * KC:(c1 + 1) * KC],
                                             qt_t, start=True, stop=True)
                            nc.vector.tensor_tensor(
                                out=sp[:, c0 * QG:(c0 + 2) * QG],
                                in0=pst, in1=hd_t, op=OP.subtract)

                    # -- prologue for the next unit: pre-sqrt its first pairs
                    #    while the sqrt table is still loaded --
                    if PRE_PAIRS and ui + 1 < len(UNITS):
                        g_nxt = UNITS[ui + 1][0]
                        pre_dma[g_nxt] = group_dmas(g_nxt)
                        qa_nxt = pre_dma[g_nxt][2]
                        for p in range(PRE_PAIRS):
                            hd_t = hd_p.tile([KC, 2 * QG], F32, tag="hdpre",
                                             bufs=PRE_PAIRS)
                            for ci, c in ((0, 2 * p), (1, 2 * p + 1)):
                                do_sqrt(hd_t[:, ci * QG:(ci + 1) * QG],
                                        c, qa_nxt)
                            pre_hd[(g_nxt, p)] = hd_t

                    # ---------- phase B: exp -> attn@V -> normalize -> out ----------
                    for g in gs:
                        q0 = g * QG
                        sp = spre[g]
                        pt = ps_pt.tile([FEAT, QG], F32, tag="pt")
                        sm = ps_sm.tile([KC, QG], F32, tag="sm")
                        for e in range(4):  # exp over 8 chunks at a time
                            lo, hi = e * 8 * QG, (e + 1) * 8 * QG
                            exp_i = nc.scalar.activation(sp[:, lo:hi],
                                                         sp[:, lo:hi], AF.Exp)
                            if last_sqrt[0] is not None:
                                add_dep_helper(exp_i.ins, last_sqrt[0],
                                               reason="ACT table phase order")
                            last_exp[0] = exp_i.ins
                            for c in range(e * 8, (e + 1) * 8):
                                nc.tensor.matmul(
                                    pt, v_t[:, c * FEAT:(c + 1) * FEAT],
                                    sp[:, c * QG:(c + 1) * QG],
                                    start=(c == 0), stop=(c == CHUNKS - 1))
                                nc.tensor.matmul(
                                    sm[0:1, :], ones16,
                                    sp[:, c * QG:(c + 1) * QG],
                                    start=(c == 0), stop=(c == CHUNKS - 1))
                        rs_t = gout_p.tile([1, QG], F32, tag="rs")
                        nc.vector.reciprocal(rs_t, sm[0:1, :])
                        rb_t = gout_p.tile([KC, QG], F32, tag="rb")
                        nc.gpsimd.partition_broadcast(rb_t, rs_t, channels=KC)
                        x1_t = gout_p.tile([FEAT, QG], F32, tag="x1")
                        nc.vector.tensor_tensor(out=x1_t, in0=pt, in1=rb_t,
                                                op=OP.mult)
                        xt_t = gout_p.tile([FEAT, QG], F16, tag="xt")
                        nc.vector.scalar_tensor_tensor(
                            out=xt_t, in0=x1_t, scalar=bv_t,
                            in1=dfg[g].bitcast(F32), op0=OP.add, op1=OP.add)
                        po = ps_sm.tile([KC, QG], F32, tag="sm")
                        nc.tensor.matmul(po[0:FEAT, :], wo_t, xt_t,
                                         start=True, stop=True)
                        o_t = gout_p.tile([FEAT, QG], F32, tag="o")
                        nc.vector.tensor_scalar_add(o_t, po[0:FEAT, :], bo_t)
                        nc.sync.dma_start(out=outT.ap()[b, :, q0:q0 + QG], in_=o_t)

    nc.compile()
    _NC_CACHE["nc"] = nc
    return nc


def _prep_inputs(sparse_xyz, sparse_feat, dense_xyz, dense_feat,
                 Wq, bq, Wk, bk, Wv, bv, Wo, bo):
    """Host-side layout prep: transposes, weight folding, xyz augmentation."""
    f32 = np.float32
    Wq = Wq.astype(f32) * f32(SCALE)
    bq_s = bq.astype(f32) * f32(SCALE)

    dfT = np.ascontiguousarray(dense_feat.transpose(0, 2, 1), dtype=f32)
    dxT = np.ascontiguousarray(dense_xyz.transpose(0, 2, 1), dtype=f32)
    sfT = np.ascontiguousarray(sparse_feat.transpose(0, 2, 1), dtype=f32)
    sxT = np.ascontiguousarray(sparse_xyz.transpose(0, 2, 1), dtype=f32)

    # ds = sum_d kaug[d] * qaug[d] = 0.25 * dist^2, computed as an fp16
    # matmul.  Naive [qn, 1, -2q] x [1, kn, k] augmentation cancels
    # catastrophically once inputs are rounded (negative ds -> sqrt NaN), so
    # every value is split hi/lo into two fp16 parts; fp16 x fp16 products
    # are exact in the fp32 PSUM accumulator, leaving ~3e-6 total error.
    f16, f64 = np.float16, np.float64

    def hilo(x):
        hi = x.astype(f16)
        lo = (x - hi.astype(f64)).astype(f16)
        return hi, lo

    qn = np.sum(dense_xyz.astype(f64) ** 2, axis=-1)   # [B, N2]
    kn = np.sum(sparse_xyz.astype(f64) ** 2, axis=-1)  # [B, N1]
    qnh, qnl = hilo(qn)
    knh, knl = hilo(kn)
    qch, qcl = hilo(dxT.astype(f64))                   # [B, 3, N2] each
    kch, kcl = hilo(sxT.astype(f64))
    one2 = np.ones((B, 1, N2), f16)
    one1 = np.ones((B, 1, N1), f16)
    quart2 = np.full((B, 1, N2), 0.25, f16)
    qaug = np.concatenate(
        [0.25 * qnh[:, None, :].astype(f16), 0.25 * qnl[:, None, :].astype(f16),
         quart2, quart2,
         -0.5 * qch, -0.5 * qch, -0.5 * qcl, -0.5 * qcl], axis=1).astype(f16)
    kaug = np.concatenate(
        [one1, one1, knh[:, None, :], knl[:, None, :],
         kch, kcl, kch, kcl], axis=1).astype(f16)

    common = {
        "sfT": sfT, "sxT": sxT, "kaug": kaug,
        "WqfT": np.ascontiguousarray(Wq[:, 3:].T, f32),
        "WqxT": np.ascontiguousarray(Wq[:, :3].T, f32),
        "WkfT": np.ascontiguousarray(Wk[:, 3:].T.astype(f32)),
        "WkxT": np.ascontiguousarray(Wk[:, :3].T.astype(f32)),
        "WvT": np.ascontiguousarray(Wv.T.astype(f32)),
        "WoT": np.ascontiguousarray(Wo.T.astype(np.float16)),
        "bq": bq_s.reshape(FEAT, 1),
        "bk": bk.astype(f32).reshape(FEAT, 1),
        "bv": bv.astype(f32).reshape(FEAT, 1),
        "bo": bo.astype(f32).reshape(FEAT, 1),
    }
    in_maps = []
    for c in range(NCORES):
        sl = slice(c * QPC, (c + 1) * QPC)
        m = dict(common)
        m["dfT"] = np.ascontiguousarray(dfT[:, :, sl])
        m["dxT"] = np.ascontiguousarray(dxT[:, :, sl])
        m["qaug"] = np.ascontiguousarray(qaug[:, :, sl])
        in_maps.append(m)
    return in_maps


def run_sharded(in_maps, trace=False):
    nc = _build()
    kwargs = {}
    if trace:
        kwargs = {"trace": True}
    return bass_utils.run_bass_kernel_spmd(
        nc, in_maps, core_ids=list(range(NCORES)), **kwargs)


def kernel(sparse_xyz, sparse_feat, dense_xyz, dense_feat,
           Wq, bq, Wk, bk, Wv, bv, Wo, bo):
    args = [np.asarray(a) for a in (sparse_xyz, sparse_feat, dense_xyz,
                                    dense_feat, Wq, bq, Wk, bk, Wv, bv,
                                    Wo, bo)]
    in_maps = _prep_inputs(*args)
    res = run_sharded(in_maps, trace=bool(os.environ.get("BASS_KERNEL_TRACE")))
    out = np.empty((B, N2, FEAT), dtype=np.float32)
    for c in range(NCORES):
        out[:, c * QPC:(c + 1) * QPC, :] = \
            res.results[c]["outT"].transpose(0, 2, 1)
    if os.environ.get("BASS_KERNEL_TRACE"):
        print("HW exec time:", res.exec_time_ns, "ns")
    return out
